# revision 2
# baseline (speedup 1.0000x reference)
"""Multi-head causal self-attention on 8 Trainium2 NeuronCores — v2.

Problem: x[2, 2048, 2048], 16 heads x 128 dim, causal softmax, four
2048x2048 projections (nn.Linear convention y = x @ W.T).

Sharding: head tensor-parallel. Core c owns heads {2c, 2c+1}: it computes
those heads' Q/K/V projections, per-head causal attention, and the slice of
the output projection that consumes those heads (Wo columns 256c..256c+256).
Each core emits a full-shape partial output; the host sums the 8 partials.

v2 changes vs v1 (456785 ns baseline):
  - bf16 storage end to end (x^T, weights, Q^T/K^T/V, E, O^T, staged
    output partials; host converts for free). Same PE speed per column
    (1 cycle/row) but halves all DMA traffic and SBUF footprint; measured
    end-to-end error ~4e-3 vs the 2e-2 gate.
  - weight DMAs issued on the ACT queue, x/out on the SP queue: the four
    weight loads no longer serialize in front of the first x block (the
    v1 startup stall was 30 us of PE idle).
  - softmax denominator: per-chunk ones-matmuls (34 us of PE) replaced by
    a Pool-engine E_sum accumulation chain (bf16 adds, idle engine) plus
    ONE [1,512] ones-matmul per (head, q-block); the [128,512] broadcast
    matmul replaced by gpsimd.partition_broadcast. PE attention work drops
    to scores + AV only.
  - Q^T/K^T PSUM evictions moved from DVE to ACT (scalar.copy); DVE keeps
    mask mults, reciprocal, O^T eviction, and half the stage copies.
  - exp lookahead deepened to 3 score pairs (et pool bufs=6).
"""

from contextlib import ExitStack

import numpy as np
import ml_dtypes

import concourse.bacc as bacc
import concourse.mybir as mybir
import concourse.tile as tile
from concourse.bass_utils import run_bass_kernel_spmd

N_CORES = 8
B = 2
SEQ = 2048
H = 2048
NHEADS = 16
D = 128
HPC = NHEADS // N_CORES  # heads per core
DC = HPC * D             # per-core head dims (256)
QB = 512                 # q/token block (moving free dim)
KTH = H // 128           # 16 contraction tiles over hidden
SCALE = 1.0 / float(np.sqrt(D))

F32 = mybir.dt.float32
BF16 = mybir.dt.bfloat16

EXP = mybir.ActivationFunctionType.Exp


def build(seq=SEQ, reps=1, only=None, bufs=None):
    """Emit the per-core program. seq is parameterized for small dev runs."""
    t = B * seq
    nblocks = seq // QB          # token blocks per batch
    nchunks = seq // 128         # 128-token chunks per batch

    nc = bacc.Bacc("TRN2", target_bir_lowering=False, debug=False,
                   num_devices=N_CORES)
    # Weights arrive pre-transposed and bf16-cast from the host:
    # w{q,k,v}t = W_c.T [H, DC], wot = Wo_c.T [DC, H], xt = x^T [H, t].
    xt_ap = nc.dram_tensor("xt_in", [H, t], BF16, kind="ExternalInput").ap()
    wqt_ap = nc.dram_tensor("wqt", [H, DC], BF16, kind="ExternalInput").ap()
    wkt_ap = nc.dram_tensor("wkt", [H, DC], BF16, kind="ExternalInput").ap()
    wvt_ap = nc.dram_tensor("wvt", [H, DC], BF16, kind="ExternalInput").ap()
    wot_ap = nc.dram_tensor("wot", [DC, H], BF16, kind="ExternalInput").ap()
    out_ap = nc.dram_tensor("out", [t, H], BF16, kind="ExternalOutput").ap()

    with tile.TileContext(nc) as tc, ExitStack() as ctx:
        const = ctx.enter_context(tc.tile_pool(name="const", bufs=1))
        # ones MATRIX: z matmul with lhsT=ones[128,128] writes the partition
        # sum to EVERY output partition — denominator + broadcast in one
        # 213ns matmul.
        ones_f32 = const.tile([128, 128], F32, name="ones_f32")
        nc.gpsimd.memset(ones_f32[:], 1.0)
        ones = const.tile([128, 128], BF16, name="ones")
        nc.vector.tensor_copy(ones[:], ones_f32[:])
        # Multiplicative causal mask: maskw[p, w] = 1 if (w - p - 384) >= 0
        # else 0. Diagonal chunk with m = kc-(n_kc-4) uses cols
        # [384-128m, +512); applied to E^T after exp.
        maskw_f32 = const.tile([128, 896], F32, name="maskw_f32")
        nc.gpsimd.memset(maskw_f32[:], 1.0)
        nc.gpsimd.affine_select(
            out=maskw_f32[:], in_=maskw_f32[:],
            compare_op=mybir.AluOpType.is_ge,
            fill=0.0, base=-384,
            pattern=[[1, 896]], channel_multiplier=-1,
        )
        maskw = const.tile([128, 896], BF16, name="maskw")
        nc.vector.tensor_copy(maskw[:], maskw_f32[:])

        # --- resident weights ---
        wt_pool = ctx.enter_context(tc.tile_pool(name="wt", bufs=1))
        wqkvT = {
            nm: wt_pool.tile([128, KTH * DC], BF16, name=f"w{nm}T", tag=f"w{nm}T")
            for nm in ("q", "k", "v")
        }
        woT = wt_pool.tile([128, HPC * H], BF16, name="woT", tag="woT")

        # PSUM: 4 one-bank slots + 2 two-bank slots = 8 banks
        pspool = ctx.enter_context(tc.tile_pool(name="ps", bufs=4, space="PSUM"))
        ps2pool = ctx.enter_context(tc.tile_pool(name="ps2", bufs=2,
                                                 space="PSUM"))

        # Weight loads: the first wq chunk rides the SP queue AHEAD of the
        # x loads (it gates the very first matmul); everything else goes on
        # the ACT queue so weights never serialize in front of x.
        wq_v = wqkvT["q"][:].rearrange("p (k dc) -> p k dc", dc=DC)
        nc.sync.dma_start(
            wq_v[:, 0:4, :],
            wqt_ap[0:512, :].rearrange("(k p) c -> p k c", p=128))
        nc.scalar.dma_start(
            wq_v[:, 4:KTH, :],
            wqt_ap[512:KTH * 128, :].rearrange("(k p) c -> p k c", p=128))
        for nm, w_ap in (("k", wkt_ap), ("v", wvt_ap)):
            wt_v = wqkvT[nm][:].rearrange("p (k dc) -> p k dc", dc=DC)
            for hf in range(2):
                k0 = hf * (KTH // 2)
                nc.scalar.dma_start(
                    wt_v[:, k0:k0 + KTH // 2, :],
                    w_ap[k0 * 128:(k0 + KTH // 2) * 128, :].rearrange(
                        "(k p) c -> p k c", p=128))
        nc.scalar.dma_start(
            woT[:].rearrange("p (hh o) -> p hh o", o=H),
            wot_ap.rearrange("(hh p) o -> p hh o", p=128))

        bd = {"et": 8, "xt": 4, "otb": 2, "stage": 6, "esum": 2,
              "look": 4}
        bd.update(bufs or {})
        qkv_pool = ctx.enter_context(tc.tile_pool(name="qkv", bufs=2))
        xt_pool = ctx.enter_context(tc.tile_pool(name="xt", bufs=bd["xt"]))
        et_pool = ctx.enter_context(tc.tile_pool(name="et", bufs=bd["et"]))
        ot_pool = ctx.enter_context(tc.tile_pool(name="otb", bufs=bd["otb"]))
        stage_pool = ctx.enter_context(
            tc.tile_pool(name="stage", bufs=bd["stage"]))
        esum_pool = ctx.enter_context(
            tc.tile_pool(name="esum", bufs=bd["esum"]))
        look = bd["look"]

        def body():
            for b in range(B):
                qt_sb = [qkv_pool.tile([128, seq], BF16, tag=f"qt{h}",
                                       name=f"qt{h}") for h in range(HPC)]
                kt_sb = [qkv_pool.tile([128, seq], BF16, tag=f"kt{h}",
                                       name=f"kt{h}") for h in range(HPC)]
                vn_sb = qkv_pool.tile([128, nchunks * DC], BF16, tag="vn",
                                      name="vn")
                if only == 'attn':
                    for tb in qt_sb + kt_sb + [vn_sb]:
                        nc.vector.memset(tb[:], 0.0)

                # phase 1: Q^T/K^T [d, tok], V [tok, d] for one block.
                def proj_block(nb):
                    tok0 = b * seq + nb * QB
                    halves = [xt_pool.tile([128, KTH // 2 * QB], BF16,
                                           tag="xt", name="xt")
                              for _ in range(2)]
                    xts = [halves[kt // (KTH // 2)]
                           [:, (kt % (KTH // 2)) * QB:
                            (kt % (KTH // 2) + 1) * QB]
                           for kt in range(KTH)]
                    # First block of the run: fine-grained pieces so the
                    # first Q matmuls start as soon as kt chunk 0 lands.
                    kpp = 2 if (b == 0 and nb == 0) else 4
                    for hf in range(2):
                        xt_v = halves[hf][:].rearrange(
                            "p (k t) -> p k t", t=QB)
                        for hq in range(8 // kpp):
                            kt0 = hf * 8 + hq * kpp
                            nc.sync.dma_start(
                                xt_v[:, hq * kpp:(hq + 1) * kpp, :],
                                xt_ap[kt0 * 128:(kt0 + kpp) * 128,
                                      tok0:tok0 + QB].rearrange(
                                          "(k p) t -> p k t", p=128))
                    for nm_p, dst in (("q", qt_sb), ("k", kt_sb)):
                        acc = [pspool.tile([128, QB], F32, tag="ps",
                                           name="pacc") for _ in range(HPC)]
                        for kt in range(KTH):
                            first, last = kt == 0, kt == KTH - 1
                            for hh in range(HPC):
                                col = kt * DC + hh * 128
                                nc.tensor.matmul(
                                    acc[hh][:],
                                    (wqkvT[nm_p][:, col:col + 128]),
                                    (xts[kt][:]), start=first, stop=last)
                        for hh in range(HPC):
                            nc.scalar.copy(
                                dst[hh][:, nb * QB:(nb + 1) * QB], acc[hh][:])
                    for c4 in range(QB // 128):
                        vn_ps = pspool.tile([128, DC], F32, tag="ps",
                                            name="vnps")
                        for kt in range(KTH):
                            nc.tensor.matmul(
                                vn_ps[:],
                                (xts[kt][:, c4 * 128:(c4 + 1) * 128]),
                                (wqkvT["v"][:, kt * DC:(kt + 1) * DC]),
                                start=(kt == 0), stop=(kt == KTH - 1))
                        chunk = nb * (QB // 128) + c4
                        nc.scalar.copy(
                            vn_sb[:, chunk * DC:(chunk + 1) * DC], vn_ps[:])

                # phase 2+3: attention + output projection for one q block
                def attn_block(qb):
                    q0 = qb * QB
                    n_kc = (qb + 1) * (QB // 128)
                    n_pc = n_kc // 2
                    ot_sbs = []
                    for hh in range(HPC):
                        esum = esum_pool.tile([128, QB], BF16,
                                              tag=f"esum{hh}", name="esum")

                        def score_pair(pc):
                            st_ps = ps2pool.tile([128, 2 * QB], F32,
                                                 tag="ps2", name="stps")
                            et2 = et_pool.tile([128, 2 * QB], BF16, tag="et",
                                               name="et")
                            for half in range(2):
                                kc = 2 * pc + half
                                # Diagonal chunks: cols < 128m are entirely
                                # non-causal — skip them in the score matmul.
                                # The stale PSUM left behind is bounded, and
                                # the mask multiply below zeroes exp(stale).
                                m = kc - (n_kc - 4)
                                w0 = 128 * m if m > 0 else 0
                                nc.tensor.matmul(
                                    st_ps[:, half * QB + w0:
                                          (half + 1) * QB],
                                    (kt_sb[hh][:, kc * 128:(kc + 1) * 128]),
                                    (qt_sb[hh][:, q0 + w0:q0 + QB]),
                                    start=True, stop=True)
                            if 2 * pc - (n_kc - 4) >= 2:
                                # final diagonal pair: only cols >= 256
                                # (half0) / >= 384 (half1) are causal —
                                # shrink the exp to those windows.
                                nc.scalar.activation(
                                    et2[:, 256:QB], st_ps[:, 256:QB],
                                    EXP, scale=SCALE)
                                nc.scalar.activation(
                                    et2[:, QB + 384:2 * QB],
                                    st_ps[:, QB + 384:2 * QB],
                                    EXP, scale=SCALE)
                            else:
                                nc.scalar.activation(et2[:], st_ps[:], EXP,
                                                     scale=SCALE)
                            for half in range(2):
                                kc = 2 * pc + half
                                m = kc - (n_kc - 4)
                                if m >= 0:
                                    w0 = 128 * m
                                    if w0 > 0:
                                        # exp of the skipped score cols is
                                        # garbage (stale PSUM, possibly
                                        # inf/nan) — hard-zero it.
                                        nc.gpsimd.memset(
                                            et2[:, half * QB:
                                                half * QB + w0], 0.0)
                                    # triangle window: keep col j iff j >= p
                                    sl = et2[:, half * QB + w0:
                                             half * QB + w0 + 128]
                                    nc.vector.tensor_mul(
                                        sl, sl, maskw[:, 384:512])
                            # E_sum chain on the (otherwise idle) Pool engine
                            if pc == 0:
                                nc.gpsimd.tensor_add(
                                    esum[:], et2[:, :QB], et2[:, QB:])
                            else:
                                nc.gpsimd.tensor_add(
                                    esum[:], esum[:], et2[:, :QB])
                                nc.gpsimd.tensor_add(
                                    esum[:], esum[:], et2[:, QB:])
                            return et2

                        ot_ps = pspool.tile([128, QB], F32, tag="ps",
                                            name="otps")
                        ets = {pc: score_pair(pc)
                               for pc in range(min(look, n_pc))}
                        for kc in range(n_kc):
                            pc = kc // 2
                            if kc % 2 == 0 and pc + look < n_pc:
                                ets[pc + look] = score_pair(pc + look)
                            et2 = ets[pc]
                            et = et2[:, (kc % 2) * QB:(kc % 2 + 1) * QB]
                            if kc % 2 == 1:
                                ets.pop(pc)
                            col = kc * DC + hh * 128
                            nc.tensor.matmul(
                                ot_ps[:], (vn_sb[:, col:col + 128]),
                                (et[:]),
                                start=(kc == 0), stop=(kc == n_kc - 1))
                            if kc == n_kc - 3:
                                # z a few AV matmuls early so the divide
                                # below starts right after the last AV.
                                # ones[128,128] writes the denominator row
                                # to every PSUM partition (broadcast free).
                                zb_ps = pspool.tile([128, QB], F32,
                                                    tag="ps", name="zbps")
                                nc.tensor.matmul(zb_ps[:], (ones[:]),
                                                 (esum[:]),
                                                 start=True, stop=True)
                        zbr = ot_pool.tile([128, QB], F32, tag="zbr",
                                           name="zbr")
                        nc.vector.reciprocal(zbr[:], zb_ps[:])
                        ot_sb = ot_pool.tile([128, QB], BF16, tag=f"ot{hh}",
                                             name=f"ot{hh}")
                        nc.vector.tensor_mul(ot_sb[:], ot_ps[:], zbr[:])
                        ot_sbs.append(ot_sb)
                    for c4 in range(QB // 128):
                        row0 = b * seq + q0 + c4 * 128
                        for oc in range(H // QB):
                            op_ps = pspool.tile([128, QB], F32, tag="ps",
                                                name="opps")
                            for hh in range(HPC):
                                nc.tensor.matmul(
                                    op_ps[:],
                                    (ot_sbs[hh][:,
                                                c4 * 128:(c4 + 1) * 128]),
                                    (woT[:, hh * H + oc * QB:
                                           hh * H + (oc + 1) * QB]),
                                    start=(hh == 0), stop=(hh == HPC - 1))
                            stg = stage_pool.tile([128, QB], BF16,
                                                  tag="stage", name="stg")
                            if oc % 2 == 0:
                                nc.vector.tensor_copy(stg[:], op_ps[:])
                            else:
                                nc.scalar.copy(stg[:], op_ps[:])
                            # final q-block: drain the last stores on two
                            # DMA queues so the tail isn't one serial queue
                            last = b == B - 1 and qb == nblocks - 1
                            eng = nc.scalar if (last and oc % 2) else nc.sync
                            eng.dma_start(
                                out_ap[row0:row0 + 128,
                                       oc * QB:(oc + 1) * QB],
                                stg[:])

                if only != 'attn':
                    for nb in range(nblocks):
                        proj_block(nb)
                if only != 'proj':
                    for qb in range(nblocks):
                        attn_block(qb)

        if reps == 1:
            body()
        else:
            with tc.For_i(0, reps, 1):
                body()

    nc.compile()
    return nc


def shard_inputs(x, Wq, Wk, Wv, Wo, seq=SEQ):
    t = B * seq
    bf = ml_dtypes.bfloat16
    x2t = np.ascontiguousarray(
        np.asarray(x, dtype=np.float32).reshape(t, H).T).astype(bf)
    Wq = np.asarray(Wq, dtype=np.float32)
    Wk = np.asarray(Wk, dtype=np.float32)
    Wv = np.asarray(Wv, dtype=np.float32)
    Wo = np.asarray(Wo, dtype=np.float32)
    in_maps = []
    for c in range(N_CORES):
        sl = slice(c * DC, (c + 1) * DC)
        in_maps.append({
            "xt_in": x2t,
            "wqt": np.ascontiguousarray(Wq[sl, :].T).astype(bf),
            "wkt": np.ascontiguousarray(Wk[sl, :].T).astype(bf),
            "wvt": np.ascontiguousarray(Wv[sl, :].T).astype(bf),
            "wot": np.ascontiguousarray(Wo[:, sl].T).astype(bf),
        })
    return in_maps


_cache = {}


def kernel(x, Wq, Wk, Wv, Wo):
    if "nc" not in _cache:
        _cache["nc"] = build()
    nc = _cache["nc"]
    in_maps = shard_inputs(x, Wq, Wk, Wv, Wo)
    res = run_bass_kernel_spmd(nc, in_maps, list(range(N_CORES)))
    acc = res.results[0]["out"].astype(np.float32)
    for c in range(1, N_CORES):
        acc = acc + res.results[c]["out"].astype(np.float32)
    return acc.reshape(B, SEQ, H)


# revision 3
# speedup vs baseline: 1.2436x; 1.2436x over previous
"""Multi-head causal self-attention on 8 Trainium2 NeuronCores — v2.

Problem: x[2, 2048, 2048], 16 heads x 128 dim, causal softmax, four
2048x2048 projections (nn.Linear convention y = x @ W.T).

Sharding: head tensor-parallel. Core c owns heads {2c, 2c+1}: it computes
those heads' Q/K/V projections, per-head causal attention, and the slice of
the output projection that consumes those heads (Wo columns 256c..256c+256).
Each core emits a full-shape partial output; the host sums the 8 partials.

v2 changes vs v1 (456785 ns baseline):
  - bf16 storage end to end (x^T, weights, Q^T/K^T/V, E, O^T, staged
    output partials; host converts for free). Same PE speed per column
    (1 cycle/row) but halves all DMA traffic and SBUF footprint; measured
    end-to-end error ~4e-3 vs the 2e-2 gate.
  - weight DMAs issued on the ACT queue, x/out on the SP queue: the four
    weight loads no longer serialize in front of the first x block (the
    v1 startup stall was 30 us of PE idle).
  - softmax denominator: per-chunk ones-matmuls (34 us of PE) replaced by
    a Pool-engine E_sum accumulation chain (bf16 adds, idle engine) plus
    ONE [1,512] ones-matmul per (head, q-block); the [128,512] broadcast
    matmul replaced by gpsimd.partition_broadcast. PE attention work drops
    to scores + AV only.
  - Q^T/K^T PSUM evictions moved from DVE to ACT (scalar.copy); DVE keeps
    mask mults, reciprocal, O^T eviction, and half the stage copies.
  - exp lookahead deepened to 3 score pairs (et pool bufs=6).
"""

from contextlib import ExitStack

import numpy as np
import ml_dtypes

import concourse.bacc as bacc
import concourse.mybir as mybir
import concourse.tile as tile
from concourse.bass_utils import run_bass_kernel_spmd

N_CORES = 8
B = 2
SEQ = 2048
H = 2048
NHEADS = 16
D = 128
HPC = NHEADS // N_CORES  # heads per core
DC = HPC * D             # per-core head dims (256)
QB = 512                 # q/token block (moving free dim)
KTH = H // 128           # 16 contraction tiles over hidden
SCALE = 1.0 / float(np.sqrt(D))

F32 = mybir.dt.float32
BF16 = mybir.dt.bfloat16

EXP = mybir.ActivationFunctionType.Exp


def build(seq=SEQ, reps=1, only=None, bufs=None):
    """Emit the per-core program. seq is parameterized for small dev runs."""
    t = B * seq
    nblocks = seq // QB          # token blocks per batch
    nchunks = seq // 128         # 128-token chunks per batch

    nc = bacc.Bacc("TRN2", target_bir_lowering=False, debug=False,
                   num_devices=N_CORES)
    # Weights arrive pre-transposed and bf16-cast from the host:
    # w{q,k,v}t = W_c.T [H, DC], wot = Wo_c.T [DC, H], xt = x^T [H, t].
    xt_ap = nc.dram_tensor("xt_in", [H, t], BF16, kind="ExternalInput").ap()
    wqt_ap = nc.dram_tensor("wqt", [H, DC], BF16, kind="ExternalInput").ap()
    wkt_ap = nc.dram_tensor("wkt", [H, DC], BF16, kind="ExternalInput").ap()
    wvt_ap = nc.dram_tensor("wvt", [H, DC], BF16, kind="ExternalInput").ap()
    wot_ap = nc.dram_tensor("wot", [DC, H], BF16, kind="ExternalInput").ap()
    out_ap = nc.dram_tensor("out", [t, H], BF16, kind="ExternalOutput").ap()

    with tile.TileContext(nc) as tc, ExitStack() as ctx:
        const = ctx.enter_context(tc.tile_pool(name="const", bufs=1))
        # ones MATRIX: z matmul with lhsT=ones[128,128] writes the partition
        # sum to EVERY output partition — denominator + broadcast in one
        # 213ns matmul.
        ones_f32 = const.tile([128, 128], F32, name="ones_f32")
        nc.gpsimd.memset(ones_f32[:], 1.0)
        ones = const.tile([128, 128], BF16, name="ones")
        nc.vector.tensor_copy(ones[:], ones_f32[:])
        # Multiplicative causal mask: maskw[p, w] = 1 if (w - p - 384) >= 0
        # else 0. Diagonal chunk with m = kc-(n_kc-4) uses cols
        # [384-128m, +512); applied to E^T after exp.
        maskw_f32 = const.tile([128, 896], F32, name="maskw_f32")
        nc.gpsimd.memset(maskw_f32[:], 1.0)
        nc.gpsimd.affine_select(
            out=maskw_f32[:], in_=maskw_f32[:],
            compare_op=mybir.AluOpType.is_ge,
            fill=0.0, base=-384,
            pattern=[[1, 896]], channel_multiplier=-1,
        )
        maskw = const.tile([128, 896], BF16, name="maskw")
        nc.vector.tensor_copy(maskw[:], maskw_f32[:])

        # --- resident weights ---
        wt_pool = ctx.enter_context(tc.tile_pool(name="wt", bufs=1))
        wqkvT = {
            nm: wt_pool.tile([128, KTH * DC], BF16, name=f"w{nm}T", tag=f"w{nm}T")
            for nm in ("q", "k", "v")
        }
        woT = wt_pool.tile([128, HPC * H], BF16, name="woT", tag="woT")

        # PSUM: 4 one-bank slots + 2 two-bank slots = 8 banks
        pspool = ctx.enter_context(tc.tile_pool(name="ps", bufs=4, space="PSUM"))
        ps2pool = ctx.enter_context(tc.tile_pool(name="ps2", bufs=2,
                                                 space="PSUM"))

        # Weight loads: the first wq chunk rides the SP queue AHEAD of the
        # x loads (it gates the very first matmul); everything else goes on
        # the ACT queue so weights never serialize in front of x.
        wq_v = wqkvT["q"][:].rearrange("p (k dc) -> p k dc", dc=DC)
        nc.sync.dma_start(
            wq_v[:, 0:4, :],
            wqt_ap[0:512, :].rearrange("(k p) c -> p k c", p=128))
        nc.scalar.dma_start(
            wq_v[:, 4:KTH, :],
            wqt_ap[512:KTH * 128, :].rearrange("(k p) c -> p k c", p=128))
        for nm, w_ap in (("k", wkt_ap), ("v", wvt_ap)):
            wt_v = wqkvT[nm][:].rearrange("p (k dc) -> p k dc", dc=DC)
            for hf in range(2):
                k0 = hf * (KTH // 2)
                nc.scalar.dma_start(
                    wt_v[:, k0:k0 + KTH // 2, :],
                    w_ap[k0 * 128:(k0 + KTH // 2) * 128, :].rearrange(
                        "(k p) c -> p k c", p=128))
        nc.scalar.dma_start(
            woT[:].rearrange("p (hh o) -> p hh o", o=H),
            wot_ap.rearrange("(hh p) o -> p hh o", p=128))

        bd = {"et": 8, "xt": 4, "otb": 2, "stage": 8, "esum": 2,
              "look": 4}
        bd.update(bufs or {})
        qkv_pool = ctx.enter_context(tc.tile_pool(name="qkv", bufs=2))
        xt_pool = ctx.enter_context(tc.tile_pool(name="xt", bufs=bd["xt"]))
        et_pool = ctx.enter_context(tc.tile_pool(name="et", bufs=bd["et"]))
        ot_pool = ctx.enter_context(tc.tile_pool(name="otb", bufs=bd["otb"]))
        stage_pool = ctx.enter_context(
            tc.tile_pool(name="stage", bufs=bd["stage"]))
        esum_pool = ctx.enter_context(
            tc.tile_pool(name="esum", bufs=bd["esum"]))
        look = bd["look"]

        def body():
            for b in range(B):
                qt_sb = [qkv_pool.tile([128, seq], BF16, tag=f"qt{h}",
                                       name=f"qt{h}") for h in range(HPC)]
                kt_sb = [qkv_pool.tile([128, seq], BF16, tag=f"kt{h}",
                                       name=f"kt{h}") for h in range(HPC)]
                vn_sb = qkv_pool.tile([128, nchunks * DC], BF16, tag="vn",
                                      name="vn")
                if only == 'attn':
                    for tb in qt_sb + kt_sb + [vn_sb]:
                        nc.vector.memset(tb[:], 0.0)

                # phase 1: Q^T/K^T [d, tok], V [tok, d] for one block.
                def proj_block(nb):
                    tok0 = b * seq + nb * QB
                    halves = [xt_pool.tile([128, KTH // 2 * QB], BF16,
                                           tag="xt", name="xt")
                              for _ in range(2)]
                    xts = [halves[kt // (KTH // 2)]
                           [:, (kt % (KTH // 2)) * QB:
                            (kt % (KTH // 2) + 1) * QB]
                           for kt in range(KTH)]
                    # First block of the run: fine-grained pieces so the
                    # first Q matmuls start as soon as kt chunk 0 lands.
                    kpp = 2 if (b == 0 and nb == 0) else 4
                    for hf in range(2):
                        xt_v = halves[hf][:].rearrange(
                            "p (k t) -> p k t", t=QB)
                        for hq in range(8 // kpp):
                            kt0 = hf * 8 + hq * kpp
                            nc.sync.dma_start(
                                xt_v[:, hq * kpp:(hq + 1) * kpp, :],
                                xt_ap[kt0 * 128:(kt0 + kpp) * 128,
                                      tok0:tok0 + QB].rearrange(
                                          "(k p) t -> p k t", p=128))
                    for nm_p, dst in (("q", qt_sb), ("k", kt_sb)):
                        acc = [pspool.tile([128, QB], F32, tag="ps",
                                           name="pacc") for _ in range(HPC)]
                        for kt in range(KTH):
                            first, last = kt == 0, kt == KTH - 1
                            for hh in range(HPC):
                                col = kt * DC + hh * 128
                                nc.tensor.matmul(
                                    acc[hh][:],
                                    (wqkvT[nm_p][:, col:col + 128]),
                                    (xts[kt][:]), start=first, stop=last)
                        for hh in range(HPC):
                            nc.scalar.copy(
                                dst[hh][:, nb * QB:(nb + 1) * QB], acc[hh][:])
                    for c4 in range(QB // 128):
                        vn_ps = pspool.tile([128, DC], F32, tag="ps",
                                            name="vnps")
                        for kt in range(KTH):
                            nc.tensor.matmul(
                                vn_ps[:],
                                (xts[kt][:, c4 * 128:(c4 + 1) * 128]),
                                (wqkvT["v"][:, kt * DC:(kt + 1) * DC]),
                                start=(kt == 0), stop=(kt == KTH - 1))
                        chunk = nb * (QB // 128) + c4
                        nc.scalar.copy(
                            vn_sb[:, chunk * DC:(chunk + 1) * DC], vn_ps[:])

                # phase 2+3: attention + output projection for one q block
                def attn_block(qb):
                    q0 = qb * QB
                    n_kc = (qb + 1) * (QB // 128)
                    n_pc = n_kc // 2
                    ot_sbs = []
                    for hh in range(HPC):
                        esum = esum_pool.tile([128, QB], BF16,
                                              tag=f"esum{hh}", name="esum")

                        def score_pair(pc):
                            st_ps = ps2pool.tile([128, 2 * QB], F32,
                                                 tag="ps2", name="stps")
                            et2 = et_pool.tile([128, 2 * QB], BF16, tag="et",
                                               name="et")
                            for half in range(2):
                                kc = 2 * pc + half
                                # Diagonal chunks: cols < 128m are entirely
                                # non-causal — skip them in the score matmul.
                                # The stale PSUM left behind is bounded, and
                                # the mask multiply below zeroes exp(stale).
                                m = kc - (n_kc - 4)
                                w0 = 128 * m if m > 0 else 0
                                nc.tensor.matmul(
                                    st_ps[:, half * QB + w0:
                                          (half + 1) * QB],
                                    (kt_sb[hh][:, kc * 128:(kc + 1) * 128]),
                                    (qt_sb[hh][:, q0 + w0:q0 + QB]),
                                    start=True, stop=True)
                            if 2 * pc - (n_kc - 4) >= 2:
                                # final diagonal pair: only cols >= 256
                                # (half0) / >= 384 (half1) are causal —
                                # shrink the exp to those windows.
                                nc.scalar.activation(
                                    et2[:, 256:QB], st_ps[:, 256:QB],
                                    EXP, scale=SCALE)
                                nc.scalar.activation(
                                    et2[:, QB + 384:2 * QB],
                                    st_ps[:, QB + 384:2 * QB],
                                    EXP, scale=SCALE)
                            else:
                                nc.scalar.activation(et2[:], st_ps[:], EXP,
                                                     scale=SCALE)
                            for half in range(2):
                                kc = 2 * pc + half
                                m = kc - (n_kc - 4)
                                if m >= 0:
                                    w0 = 128 * m
                                    if w0 > 0:
                                        # exp of the skipped score cols is
                                        # garbage (stale PSUM, possibly
                                        # inf/nan) — hard-zero it.
                                        nc.gpsimd.memset(
                                            et2[:, half * QB:
                                                half * QB + w0], 0.0)
                                    # triangle window: keep col j iff j >= p
                                    sl = et2[:, half * QB + w0:
                                             half * QB + w0 + 128]
                                    nc.vector.tensor_mul(
                                        sl, sl, maskw[:, 384:512])
                            # E_sum chain on the (otherwise idle) Pool engine
                            if pc == 0:
                                nc.gpsimd.tensor_add(
                                    esum[:], et2[:, :QB], et2[:, QB:])
                            else:
                                nc.gpsimd.tensor_add(
                                    esum[:], esum[:], et2[:, :QB])
                                nc.gpsimd.tensor_add(
                                    esum[:], esum[:], et2[:, QB:])
                            return et2

                        ot_ps = pspool.tile([128, QB], F32, tag="ps",
                                            name="otps")
                        ets = {pc: score_pair(pc)
                               for pc in range(min(look, n_pc))}
                        for kc in range(n_kc):
                            pc = kc // 2
                            if kc % 2 == 0 and pc + look < n_pc:
                                ets[pc + look] = score_pair(pc + look)
                            et2 = ets[pc]
                            et = et2[:, (kc % 2) * QB:(kc % 2 + 1) * QB]
                            if kc % 2 == 1:
                                ets.pop(pc)
                            col = kc * DC + hh * 128
                            nc.tensor.matmul(
                                ot_ps[:], (vn_sb[:, col:col + 128]),
                                (et[:]),
                                start=(kc == 0), stop=(kc == n_kc - 1))
                            if kc == n_kc - 3:
                                # z a few AV matmuls early so the divide
                                # below starts right after the last AV.
                                # ones[128,128] writes the denominator row
                                # to every PSUM partition (broadcast free).
                                zb_ps = pspool.tile([128, QB], F32,
                                                    tag="ps", name="zbps")
                                nc.tensor.matmul(zb_ps[:], (ones[:]),
                                                 (esum[:]),
                                                 start=True, stop=True)
                        zbr = ot_pool.tile([128, QB], F32, tag="zbr",
                                           name="zbr")
                        nc.vector.reciprocal(zbr[:], zb_ps[:])
                        ot_sb = ot_pool.tile([128, QB], BF16, tag=f"ot{hh}",
                                             name=f"ot{hh}")
                        nc.vector.tensor_mul(ot_sb[:], ot_ps[:], zbr[:])
                        ot_sbs.append(ot_sb)
                    for c4 in range(QB // 128):
                        row0 = b * seq + q0 + c4 * 128
                        for oc in range(H // QB):
                            op_ps = pspool.tile([128, QB], F32, tag="ps",
                                                name="opps")
                            for hh in range(HPC):
                                nc.tensor.matmul(
                                    op_ps[:],
                                    (ot_sbs[hh][:,
                                                c4 * 128:(c4 + 1) * 128]),
                                    (woT[:, hh * H + oc * QB:
                                           hh * H + (oc + 1) * QB]),
                                    start=(hh == 0), stop=(hh == HPC - 1))
                            stg = stage_pool.tile([128, QB], BF16,
                                                  tag="stage", name="stg")
                            if oc % 2 == 0:
                                nc.vector.tensor_copy(stg[:], op_ps[:])
                            else:
                                nc.scalar.copy(stg[:], op_ps[:])
                            # final q-block: drain the last stores on two
                            # DMA queues so the tail isn't one serial queue
                            last = b == B - 1 and qb == nblocks - 1
                            eng = nc.scalar if (last and oc % 2) else nc.sync
                            eng.dma_start(
                                out_ap[row0:row0 + 128,
                                       oc * QB:(oc + 1) * QB],
                                stg[:])

                if only != 'attn':
                    for nb in range(nblocks):
                        proj_block(nb)
                if only != 'proj':
                    for qb in range(nblocks):
                        attn_block(qb)

        if reps == 1:
            body()
        else:
            with tc.For_i(0, reps, 1):
                body()

    nc.compile()
    return nc


def shard_inputs(x, Wq, Wk, Wv, Wo, seq=SEQ):
    t = B * seq
    bf = ml_dtypes.bfloat16
    x2t = np.ascontiguousarray(
        np.asarray(x, dtype=np.float32).reshape(t, H).T).astype(bf)
    Wq = np.asarray(Wq, dtype=np.float32)
    Wk = np.asarray(Wk, dtype=np.float32)
    Wv = np.asarray(Wv, dtype=np.float32)
    Wo = np.asarray(Wo, dtype=np.float32)
    in_maps = []
    for c in range(N_CORES):
        sl = slice(c * DC, (c + 1) * DC)
        in_maps.append({
            "xt_in": x2t,
            "wqt": np.ascontiguousarray(Wq[sl, :].T).astype(bf),
            "wkt": np.ascontiguousarray(Wk[sl, :].T).astype(bf),
            "wvt": np.ascontiguousarray(Wv[sl, :].T).astype(bf),
            "wot": np.ascontiguousarray(Wo[:, sl].T).astype(bf),
        })
    return in_maps


_cache = {}


def kernel(x, Wq, Wk, Wv, Wo):
    if "nc" not in _cache:
        _cache["nc"] = build()
    nc = _cache["nc"]
    in_maps = shard_inputs(x, Wq, Wk, Wv, Wo)
    res = run_bass_kernel_spmd(nc, in_maps, list(range(N_CORES)))
    acc = res.results[0]["out"].astype(np.float32)
    for c in range(1, N_CORES):
        acc = acc + res.results[c]["out"].astype(np.float32)
    return acc.reshape(B, SEQ, H)
